# revision 14
# baseline (speedup 1.0000x reference)
"""Distributed multi-head attention kernel for one TRN2 chip (8 NeuronCores).

Problem: b=2, n=2048, dim=1024, heads=16, hd=64.
  qkv = x @ Wqkv.T  (qkv-major split) -> RoPE(q,k) -> softmax(q k^T/8) v
  -> merge heads -> @ Wproj.T + bproj

Sharding: each core owns 2 heads (of 16) for BOTH batches. QKV projection,
RoPE and attention are fully head-local. Five 8-way AllToAlls (b0 halves,
b1 half + two quarters) redistribute attention outputs head-major ->
token-major as each span is staged, so only a quarter-size exchange + a
64-token projection sit in the tail; a dummy warm-up AllToAll at t=0 absorbs
the ~25us first-collective cost. Core c outputs, per batch: b0 tokens
[128c,+128) and [1024+128c,+128); b1 tokens [128c,+128), [1024+64c,+64),
[1536+64c,+64). Host reassembles.

Per-core inputs (see make_in_maps) are pre-transposed/pre-cast on the host so
no DMA-xbar transposes and no on-device sin/cos prep are needed:
  x        [1024, 4096] bf16  x^T: channels x flat tokens
  wqkv     [1024, 384]  bf16  (q|k|v rows for my heads)^T
  wproj    [1024, 1024] bf16  Wproj^T: [d', f]
  biasbf   [1, 1024]    bf16
  sin2     [128, 16*64] bf16  sin[128*pt+p, d]; broadcast-AP over q|k copies
  cos2     [128, 16*64] bf16
  sneg2    [128, 16*64] bf16  -sin on d<32, +sin on d>=32 (rotate_half)
  ident    [128, 128]   bf16  identity for PE transposes
  out      [512, 1024]  f32   row blocks per (batch, span), see OUT_ROWS

All matmuls bf16 (PSUM accumulates f32). scoresT layout [k_j, q_i] (k
stationary, both heads row-packed across the 128 partitions) so softmax needs
no transposes: denominators come from a ones-column appended to v. exp on
ScalarE with fused 1/8 scale; no max subtraction (scores std ~2). The QKV
chain for each batch is software-pipelined into that batch's first
attention q-block (lag 4 tiles). Projections are interleaved 2 matmuls per
attention iteration into b1's exp-bound q-blocks (a contiguous block would
starve the FIFO exp stream); their A2A-result fetches ride the GpSimd DMA
queue so head-of-line waits never block staging.
"""

import os
import numpy as np

NUM_CORES = 8
B, N, DIM, NH, HD = 2, 2048, 1024, 16, 64
T = B * N                 # 4096 flat tokens
HPC = NH // NUM_CORES     # 2 heads per core
P = 128
CT = DIM // P             # 8 channel tiles
QW = HPC * HD             # 128
FQKV = 3 * QW             # 384
QB = 512                  # attention q-block width
TTH = N // P              # 16 token tiles per batch
HT = N // 2               # 1024 tokens per half

# A2A spans: key -> (batch, first token, tokens per core)
A2A_SPANS = {
    (0, 0): (0, 0, P), (0, 1): (0, HT, P),
    (1, 0): (1, 0, P), (1, 2): (1, HT, 64), (1, 3): (1, HT + 512, 64),
}
OUT_ROWS = {(0, 0): 0, (0, 1): 128, (1, 0): 256, (1, 2): 384, (1, 3): 448}

_CACHE = {}


def _build_nc():
    from concourse import bacc, mybir, tile
    from concourse.bass import broadcast_tensor_aps

    f32 = mybir.dt.float32
    bf16 = mybir.dt.bfloat16
    Exp = mybir.ActivationFunctionType.Exp
    mult = mybir.AluOpType.mult
    add = mybir.AluOpType.add

    nc = bacc.Bacc("TRN2", target_bir_lowering=False, debug=False,
                   num_devices=NUM_CORES)

    x_d = nc.dram_tensor("x", [DIM, T], bf16, kind="ExternalInput")
    wqkv_d = nc.dram_tensor("wqkv", [DIM, FQKV], bf16, kind="ExternalInput")
    wproj_d = nc.dram_tensor("wproj", [DIM, DIM], bf16, kind="ExternalInput")
    bias_d = nc.dram_tensor("biasbf", [1, DIM], bf16, kind="ExternalInput")
    sin2_d = nc.dram_tensor("sin2", [P, 16 * HD], bf16, kind="ExternalInput")
    cos2_d = nc.dram_tensor("cos2", [P, 16 * HD], bf16, kind="ExternalInput")
    sneg2_d = nc.dram_tensor("sneg2", [P, 16 * HD], bf16, kind="ExternalInput")
    ident_d = nc.dram_tensor("ident", [P, P], bf16, kind="ExternalInput")
    out_d = nc.dram_tensor("out", [4 * P, DIM], f32, kind="ExternalOutput")
    a2a_in = {k: nc.dram_tensor(f"a2a_in{k[0]}_{k[1]}",
                                [NUM_CORES * P, tpc], bf16)
              for k, (_, _, tpc) in A2A_SPANS.items()}
    a2a_out = {k: nc.dram_tensor(f"a2a_out{k[0]}_{k[1]}",
                                 [NUM_CORES * P, tpc], bf16)
               for k, (_, _, tpc) in A2A_SPANS.items()}
    warm_in = nc.dram_tensor("warm_in", [P, 16], bf16)
    warm_out = nc.dram_tensor("warm_out", [P, 16], bf16)

    with tile.TileContext(nc) as tc:
        with (
            tc.tile_pool(name="persist", bufs=1) as pers,
            tc.tile_pool(name="work", bufs=3) as wp,
            tc.tile_pool(name="expp", bufs=4) as ep,
            tc.tile_pool(name="psA", bufs=2, space="PSUM") as psA,   # qkv/bc/proj/tp
            tc.tile_pool(name="psS", bufs=4, space="PSUM") as psS,   # scores
            tc.tile_pool(name="psV", bufs=1, space="PSUM") as psV,   # av accum
        ):
            # ---------------- persistent SBUF ----------------
            wqkvT = pers.tile([P, CT * FQKV], bf16)     # ct-block: [128c, 384f]
            wprojT = pers.tile([P, CT * DIM], bf16)     # dt-block: [128d', 1024f]
            xT = pers.tile([P, CT * T], bf16)           # ct-block: [128c, 4096t]
            qT = pers.tile([P, T], bf16)                # [d(2 heads), flat t]
            kT = pers.tile([P, T], bf16)
            v_sb = pers.tile([P, HPC * (T // P) * 65], bf16)
            aoT = pers.tile([P, T], bf16)               # [d', flat t]
            # A2A results: one tile per span so a later span's fetch never
            # creates a false write->read dependency for an earlier proj
            aoTr = {k: pers.tile([P, NUM_CORES * tpc], bf16,
                                 name=f"aoTr{k[0]}_{k[1]}",
                                 tag=f"aoTr{k[0]}_{k[1]}")
                    for k, (_, _, tpc) in A2A_SPANS.items()}
            sin2 = pers.tile([P, 16 * HD], bf16)
            cos2 = pers.tile([P, 16 * HD], bf16)
            sneg2 = pers.tile([P, 16 * HD], bf16)
            ones_col = pers.tile([1, P], bf16)
            bias_bf = pers.tile([1, DIM], bf16)
            ident = pers.tile([P, P], bf16)

            nc.vector.memset(ones_col, 1.0)
            nc.vector.memset(v_sb, 1.0)                 # ones cols survive

            # warm up the collectives path while DMAs stream in
            wtile = wp.tile([P, 16], bf16, tag="warm", bufs=1)
            nc.vector.memset(wtile, 0.0)
            nc.sync.dma_start(warm_in.ap(), wtile)
            nc.gpsimd.collective_compute(
                "AllToAll", mybir.AluOpType.bypass,
                replica_groups=[list(range(NUM_CORES))],
                ins=[warm_in.ap().opt()], outs=[warm_out.ap().opt()])

            # ---------------- prep loads (no xbar transposes) ----------------
            # scalar queue: wqkv, sincos, ident, bias (in need order);
            # sync queue: xT finest-grained first so tile-0's QKV matmuls
            # start as early as possible
            for ct in range(CT):
                nc.scalar.dma_start(wqkvT[:, FQKV * ct:FQKV * (ct + 1)],
                                    wqkv_d[P * ct:P * (ct + 1), :])
            nc.scalar.dma_start(sin2, sin2_d.ap())
            nc.scalar.dma_start(cos2, cos2_d.ap())
            nc.scalar.dma_start(sneg2, sneg2_d.ap())
            nc.scalar.dma_start(ident, ident_d.ap())
            nc.scalar.dma_start(bias_bf, bias_d.ap())
            for lo, hi in ((0, 128), (128, 512), (512, 1024),
                           (1024, 2048), (2048, 3072), (3072, 4096)):
                for ct in range(CT):
                    nc.sync.dma_start(
                        xT[:, T * ct + lo:T * ct + hi],
                        x_d[P * ct:P * (ct + 1), lo:hi])

            s1 = sin2.rearrange("p (pt c d) -> p pt c d", pt=16, c=1)
            c1 = cos2.rearrange("p (pt c d) -> p pt c d", pt=16, c=1)
            n1 = sneg2.rearrange("p (pt c d) -> p pt c d", pt=16, c=1)

            def bc4(src, like):
                """Broadcast a [p, 1, w] AP over the 4 q|k copies."""
                a, _ = broadcast_tensor_aps(src, like)
                return a

            def emit_qkv_tile(b, tt, act_copies=False):
                """QKV matmul + RoPE + PE transposes for one 128-token tile.

                act_copies: route PSUM->SBUF copies to ScalarE (only safe in
                windows where the exp stream has slack, i.e. b0's first
                q-block)."""
                cp = nc.scalar.copy if act_copies else nc.vector.tensor_copy
                ftt = TTH * b + tt
                qkvp = psA.tile([P, 512], f32, tag="mm", name="qkvp")
                for ct in range(CT):
                    base = T * ct + N * b
                    nc.tensor.matmul(
                        qkvp[:, 0:FQKV],
                        xT[:, base + P * tt:base + P * (tt + 1)],
                        wqkvT[:, FQKV * ct:FQKV * (ct + 1)],
                        start=(ct == 0), stop=(ct == CT - 1))
                qkc = wp.tile([P, 2 * QW], bf16, tag="qkc")
                cp(qkc, qkvp[:, 0:2 * QW])
                pt = tt % 16
                qk3 = qkc.rearrange("p (c d) -> p c d", c=4)
                t1 = wp.tile([P, 2 * QW], bf16, tag="t1")
                t13 = t1.rearrange("p (c d) -> p c d", c=4)
                nc.vector.tensor_tensor(
                    t13[:, :, 0:32], qk3[:, :, 32:64],
                    bc4(n1[:, pt, :, 0:32], qk3[:, :, 32:64]), mult)
                nc.vector.tensor_tensor(
                    t13[:, :, 32:64], qk3[:, :, 0:32],
                    bc4(n1[:, pt, :, 32:64], qk3[:, :, 0:32]), mult)
                qkcos = wp.tile([P, 2 * QW], bf16, tag="qkcos")
                qkcos3 = qkcos.rearrange("p (c d) -> p c d", c=4)
                nc.vector.tensor_tensor(
                    qkcos3, qk3, bc4(c1[:, pt, :, :], qk3), mult)
                qrope = wp.tile([P, QW], bf16, tag="qrope")
                krope = wp.tile([P, QW], bf16, tag="krope")
                nc.vector.tensor_tensor(qrope, qkcos[:, 0:QW], t1[:, 0:QW], add)
                nc.vector.tensor_tensor(krope, qkcos[:, QW:2 * QW],
                                        t1[:, QW:2 * QW], add)
                # in qq0 the scores slots are mostly idle; borrowing them
                # for the transpose tile lets qkvp double-buffer on psA
                tpool, ttag = (psS, "scores") if act_copies else (psA, "mm")
                tp = tpool.tile([P, 2 * P], bf16, tag=ttag, name="tp")
                nc.tensor.transpose(tp[:, 0:P], qrope, ident)
                nc.tensor.transpose(tp[:, P:2 * P], krope, ident)
                cp(qT[:, P * ftt:P * (ftt + 1)], tp[:, 0:P])
                cp(kT[:, P * ftt:P * (ftt + 1)], tp[:, P:2 * P])
                vv = v_sb.rearrange("p (h t e) -> p h t e", h=HPC, t=T // P)
                nc.vector.tensor_copy(
                    vv[:, :, ftt, 0:HD],
                    qkvp[:, 2 * QW:3 * QW].rearrange("p (h d) -> p h d", h=HPC))

            def emit_scores_exp(b, qq, jt):
                """Scores + exp for one (q-block, j-tile); returns per-head exp
                tiles. Separate per-head score banks (4 bufs) give the exp
                stream ~2 iterations of lookahead."""
                ftt = TTH * b + jt
                ets = []
                for h in range(HPC):
                    sp = psS.tile([P, QB], f32, tag="scores", name="sp")
                    nc.tensor.matmul(
                        sp,
                        kT[HD * h:HD * (h + 1), P * ftt:P * (ftt + 1)],
                        qT[HD * h:HD * (h + 1),
                           N * b + QB * qq:N * b + QB * (qq + 1)],
                        start=True, stop=True)
                    et = ep.tile([P, QB], bf16, tag="expT", bufs=8, name="et")
                    nc.scalar.activation(et, sp, Exp, scale=float(HD) ** -0.5)
                    ets.append(et)
                return ets

            def emit_av(b, qq, jt, av, ets):
                ftt = TTH * b + jt
                for h in range(HPC):
                    blk = (h * (T // P) + ftt) * 65
                    nc.tensor.matmul(av[h], v_sb[:, blk:blk + 65],
                                     ets[h],
                                     start=(jt == 0), stop=(jt == TTH - 1))

            def emit_avf(avp):
                avf = wp.tile([65, HPC * QB], f32, tag="avf", bufs=2, name="avf")
                nc.vector.tensor_copy(avf, avp)
                return avf

            def emit_norm_rest(b, qq, avf):
                """Denominator broadcast + reciprocal + normalize, plus the A2A
                staging of this q-block's chunks."""
                for h in range(HPC):
                    sums = wp.tile([1, QB], bf16, tag="sums", name="sums")
                    nc.vector.tensor_copy(sums, avf[64:65, QB * h:QB * (h + 1)])
                    bc = psA.tile([64, QB], f32, tag="mm", name="bc")
                    nc.tensor.matmul(bc, ones_col[:, 0:64], sums,
                                     start=True, stop=True)
                    rc = wp.tile([64, QB], f32, tag="recip", bufs=2, name="rc")
                    nc.vector.reciprocal_approx_fast(rc, bc)
                    nc.vector.tensor_tensor(
                        aoT[HD * h:HD * (h + 1),
                            N * b + QB * qq:N * b + QB * (qq + 1)],
                        avf[0:64, QB * h:QB * (h + 1)], rc, mult)
                # stage this q-block's chunks into its span's A2A input
                if b == 0 or qq < 2:
                    key, r0, c, t = (b, qq // 2), 4 * P * (qq % 2), 4, P
                else:
                    key, r0, c, t = (1, qq), 0, 8, 64
                a2i = a2a_in[key][r0:r0 + c * P].rearrange(
                    "(c p) t -> p c t", p=P)
                nc.sync.dma_start(
                    a2i, aoT[:, N * b + QB * qq:N * b + QB * (qq + 1)].rearrange(
                        "p (c t) -> p c t", c=c))

            def emit_a2a(key):
                nc.gpsimd.collective_compute(
                    "AllToAll", mybir.AluOpType.bypass,
                    replica_groups=[list(range(NUM_CORES))],
                    ins=[a2a_in[key].ap().opt()], outs=[a2a_out[key].ap().opt()])

            def emit_fetch(key):
                """Fetch A2A result into aoTr (GpSimd DMA queue: its wait on
                the collective must not block staging on the sync queue)."""
                a2o = a2a_out[key].ap().rearrange("(c p) t -> p c t", p=P)
                nc.gpsimd.dma_start(
                    aoTr[key].rearrange("p (c t) -> p c t", c=NUM_CORES), a2o)

            def make_proj_steps(key, act_ob=False):
                """Projection of one span, split into 8 small steps (2 matmuls
                each) so interleaving never starves the exp stream. act_ob:
                route the output copy to ScalarE (safe once exp is done)."""
                _, _, tpc = A2A_SPANS[key]
                rout = OUT_ROWS[key]
                cp = nc.scalar.copy if act_ob else nc.vector.tensor_copy
                st = {}
                steps = []

                def mk(fb, i):
                    def fn():
                        if i == 0:
                            st[fb] = psA.tile([tpc, 512], f32, tag="mm",
                                              name="proj")
                        for dt in (2 * i, 2 * i + 1):
                            lo = tpc * dt
                            nc.tensor.matmul(
                                st[fb], aoTr[key][:, lo:lo + tpc],
                                wprojT[:, DIM * dt + 512 * fb:
                                       DIM * dt + 512 * (fb + 1)],
                                start=(dt == 0), stop=False)
                        if i == 3:
                            nc.tensor.matmul(
                                st[fb], ones_col[:, 0:tpc],
                                bias_bf[:, 512 * fb:512 * (fb + 1)],
                                start=False, stop=True)
                            ob = wp.tile([tpc, 512], f32, tag="ob", bufs=2,
                                         name="ob")
                            cp(ob, st[fb])
                            nc.scalar.dma_start(
                                out_d[rout:rout + tpc,
                                      512 * fb:512 * (fb + 1)], ob)
                    return fn

                for fb in range(DIM // 512):
                    for i in range(4):
                        steps.append(mk(fb, i))
                return steps

            # ---------------- main schedule ----------------
            # b0: qq0 software-pipelines b0's QKV chain (ScalarE-assisted
            # copies — exp has slack there); qq1-3 interleave b1's QKV tiles
            # (DVE copies). AV matmuls trail scores/exp by one iteration and
            # the last AV + avf copy of each q-block is carried into the next
            # block's first iteration, so the boundary never stalls the exp
            # stream. Each span's A2A fires as soon as its q-blocks are
            # staged; its projection trickles into later q-blocks' PE slack.
            LAG = 4
            state = {"pend": None, "carry": None}
            proj_q = []

            def flush_carry():
                if state["carry"] is not None:
                    state["carry"]()
                    state["carry"] = None

            def set_carry(b, qq, avp, av, et):
                def fn():
                    emit_av(b, qq, TTH - 1, av, et)
                    state["pend"] = (b, qq, emit_avf(avp))
                state["carry"] = fn

            def flush_pend():
                if state["pend"] is not None:
                    emit_norm_rest(*state["pend"])
                    state["pend"] = None

            def proj_step():
                if proj_q:
                    proj_q.pop(0)()

            for qq in range(N // QB):
                avp = psV.tile([65, HPC * QB], f32, tag="av", name="avp")
                av = [avp[:, QB * h:QB * (h + 1)] for h in range(HPC)]
                prev_et = None
                if qq == 0:
                    for step in range(TTH + LAG):
                        if step < TTH:
                            emit_qkv_tile(0, step, act_copies=True)
                        if step >= LAG:
                            jt = step - LAG
                            et = emit_scores_exp(0, qq, jt)
                            if jt >= 1:
                                emit_av(0, qq, jt - 1, av, prev_et)
                            prev_et = et
                else:
                    for jt in range(TTH):
                        et = emit_scores_exp(0, qq, jt)
                        if jt == 0:
                            flush_carry()
                        else:
                            emit_av(0, qq, jt - 1, av, prev_et)
                        prev_et = et
                        if jt == 2:
                            flush_pend()
                            if qq == 2:
                                emit_a2a((0, 0))   # b0 half 0 staged by now
                            if qq == 3:
                                emit_fetch((0, 0))
                        if jt % 3 == 0:
                            nb1 = 6 * (qq - 1) + jt // 3
                            if nb1 < TTH:
                                emit_qkv_tile(1, nb1)
                set_carry(0, qq, avp, av, prev_et)
                if qq == 1:
                    for dt in range(CT):
                        nc.sync.dma_start(wprojT[:, DIM * dt:DIM * (dt + 1)],
                                          wproj_d[P * dt:P * (dt + 1), :])
            for qq in range(N // QB):
                avp = psV.tile([65, HPC * QB], f32, tag="av", name="avp")
                av = [avp[:, QB * h:QB * (h + 1)] for h in range(HPC)]
                prev_et = None
                for jt in range(TTH):
                    et = emit_scores_exp(1, qq, jt)
                    if jt == 0:
                        flush_carry()
                    else:
                        emit_av(1, qq, jt - 1, av, prev_et)
                    prev_et = et
                    if jt == 2:
                        flush_pend()
                        if qq == 0:
                            emit_a2a((0, 1))   # b0 half 1 staged by now
                            proj_q.extend(make_proj_steps((0, 0)))
                        if qq == 1:
                            proj_q.extend(make_proj_steps((0, 1)))
                        if qq == 2:
                            emit_a2a((1, 0))   # b1 half 0 staged by now
                        if qq == 3:
                            emit_a2a((1, 2))   # b1 quarter 2 staged by now
                            proj_q.extend(make_proj_steps((1, 0)))
                    if 4 <= jt <= 11:
                        proj_step()
                    if jt == 13:
                        if qq == 0:
                            emit_fetch((0, 1))
                        if qq == 2:
                            emit_fetch((1, 0))
                        if qq == 3:
                            emit_fetch((1, 2))
                set_carry(1, qq, avp, av, prev_et)
            flush_carry()
            flush_pend()
            emit_a2a((1, 3))
            for fn in proj_q:            # leftovers, if any
                fn()
            for fn in make_proj_steps((1, 2), act_ob=True):
                fn()
            emit_fetch((1, 3))
            for fn in make_proj_steps((1, 3), act_ob=True):
                fn()

    nc.compile()
    return nc


def _get_nc():
    if "nc" not in _CACHE:
        _CACHE["nc"] = _build_nc()
    return _CACHE["nc"]


def make_in_maps(x, Wqkv, Wproj, bproj, sin, cos):
    """Shard full (f32) inputs into per-core in_maps (pre-cast + pre-transposed)."""
    import ml_dtypes
    bf16 = ml_dtypes.bfloat16
    xT = np.ascontiguousarray(
        np.asarray(x, np.float32).reshape(T, DIM).astype(bf16).T)
    Wqkv = np.asarray(Wqkv, np.float32).astype(bf16)
    WprojT = np.ascontiguousarray(np.asarray(Wproj, np.float32).astype(bf16).T)
    biasbf = np.asarray(bproj, np.float32).reshape(1, DIM).astype(bf16)
    sin = np.asarray(sin, np.float32)
    cos = np.asarray(cos, np.float32)
    # sin/cos RoPE tiles, precomputed host-side:
    #   sin2[p, 64*pt + d] = sin[128*pt + p, d]
    #   sneg2 = -sin on d<32, +sin on d>=32 (rotate_half multiplier)
    s_t = sin.reshape(16, P, HD).transpose(1, 0, 2)          # [p, pt, d]
    c_t = cos.reshape(16, P, HD).transpose(1, 0, 2)
    n_t = np.concatenate([-s_t[..., 0:32], s_t[..., 32:64]], axis=-1)
    sin2 = np.ascontiguousarray(s_t.reshape(P, 16 * HD)).astype(bf16)
    cos2 = np.ascontiguousarray(c_t.reshape(P, 16 * HD)).astype(bf16)
    sneg2 = np.ascontiguousarray(n_t.reshape(P, 16 * HD)).astype(bf16)
    ident = np.eye(P, dtype=bf16)
    in_maps = []
    for c in range(NUM_CORES):
        r = P * c
        wq = Wqkv[r:r + P]
        wk = Wqkv[DIM + r:DIM + r + P]
        wv = Wqkv[2 * DIM + r:2 * DIM + r + P]
        in_maps.append({
            "x": xT,
            "wqkv": np.ascontiguousarray(np.concatenate([wq, wk, wv], 0).T),
            "wproj": WprojT,
            "biasbf": biasbf,
            "sin2": sin2,
            "cos2": cos2,
            "sneg2": sneg2,
            "ident": ident,
        })
    return in_maps


def assemble(outs):
    """Reassemble per-core 'out' tensors (row blocks per OUT_ROWS) into the
    full [B, N, DIM] output."""
    out = np.empty((B, N, DIM), np.float32)
    for c in range(NUM_CORES):
        o = outs[c]
        for key, (b, tok0, tpc) in A2A_SPANS.items():
            r = OUT_ROWS[key]
            t0 = tok0 + tpc * c
            out[b, t0:t0 + tpc] = o[r:r + tpc]
    return out


def kernel(x, Wqkv, Wproj, bproj, sin, cos):
    from concourse.bass_utils import run_bass_kernel_spmd

    nc = _get_nc()
    in_maps = make_in_maps(x, Wqkv, Wproj, bproj, sin, cos)
    trace = bool(int(os.environ.get("KERNEL_TRACE", "0")))
    res = run_bass_kernel_spmd(nc, in_maps, core_ids=list(range(NUM_CORES)),
                               trace=trace)
    _CACHE["last_result"] = res
    return assemble([res.results[c]["out"] for c in range(NUM_CORES)])


# revision 16
# speedup vs baseline: 1.1040x; 1.1040x over previous
"""Distributed multi-head attention kernel for one TRN2 chip (8 NeuronCores).

Problem: b=2, n=2048, dim=1024, heads=16, hd=64.
  qkv = x @ Wqkv.T  (qkv-major split) -> RoPE(q,k) -> softmax(q k^T/8) v
  -> merge heads -> @ Wproj.T + bproj

Sharding: each core owns 2 heads (of 16) for BOTH batches. QKV projection,
RoPE and attention are fully head-local. Five 8-way AllToAlls (b0 halves,
b1 half + two quarters) redistribute attention outputs head-major ->
token-major as each span is staged, so only a quarter-size exchange + a
64-token projection sit in the tail; a dummy warm-up AllToAll at t=0 absorbs
the ~25us first-collective cost. Core c outputs, per batch: b0 tokens
[128c,+128) and [1024+128c,+128); b1 tokens [128c,+128), [1024+64c,+64),
[1536+64c,+64). Host reassembles.

Per-core inputs (see make_in_maps) are pre-transposed/pre-cast on the host so
no DMA-xbar transposes and no on-device sin/cos prep are needed:
  x        [1024, 4096] bf16  x^T: channels x flat tokens
  wqkv     [1024, 384]  bf16  (q|k|v rows for my heads)^T
  wproj    [1024, 1024] bf16  Wproj^T: [d', f]
  biasbf   [1, 1024]    bf16
  sin2     [128, 16*64] bf16  sin[128*pt+p, d]; broadcast-AP over q|k copies
  cos2     [128, 16*64] bf16
  sneg2    [128, 16*64] bf16  -sin on d<32, +sin on d>=32 (rotate_half)
  ident    [128, 128]   bf16  identity for PE transposes
  out      [512, 1024]  f32   row blocks per (batch, span), see OUT_ROWS

All matmuls bf16 (PSUM accumulates f32). scoresT layout [k_j, q_i] (k
stationary, both heads row-packed across the 128 partitions) so softmax needs
no transposes: denominators come from a ones-column appended to v. exp on
ScalarE with fused 1/8 scale; no max subtraction (scores std ~2). The QKV
chain for each batch is software-pipelined into that batch's first
attention q-block (lag 4 tiles). Projections are interleaved 2 matmuls per
attention iteration into b1's exp-bound q-blocks (a contiguous block would
starve the FIFO exp stream); their A2A-result fetches ride the GpSimd DMA
queue so head-of-line waits never block staging.
"""

import os
import numpy as np

NUM_CORES = 8
B, N, DIM, NH, HD = 2, 2048, 1024, 16, 64
T = B * N                 # 4096 flat tokens
HPC = NH // NUM_CORES     # 2 heads per core
P = 128
CT = DIM // P             # 8 channel tiles
QW = HPC * HD             # 128
FQKV = 3 * QW             # 384
QB = 512                  # attention q-block width
TTH = N // P              # 16 token tiles per batch
HT = N // 2               # 1024 tokens per half

# A2A spans: key -> (batch, first token, tokens per core)
A2A_SPANS = {
    (0, 0): (0, 0, P), (0, 1): (0, HT, P),
    (1, 0): (1, 0, P), (1, 2): (1, HT, 64), (1, 3): (1, HT + 512, 64),
}
OUT_ROWS = {(0, 0): 0, (0, 1): 128, (1, 0): 256, (1, 2): 384, (1, 3): 448}

_CACHE = {}


def _build_nc():
    from concourse import bacc, mybir, tile
    from concourse.bass import broadcast_tensor_aps

    f32 = mybir.dt.float32
    bf16 = mybir.dt.bfloat16
    Exp = mybir.ActivationFunctionType.Exp
    mult = mybir.AluOpType.mult
    add = mybir.AluOpType.add

    nc = bacc.Bacc("TRN2", target_bir_lowering=False, debug=False,
                   num_devices=NUM_CORES)

    x_d = nc.dram_tensor("x", [DIM, T], bf16, kind="ExternalInput")
    wqkv_d = nc.dram_tensor("wqkv", [DIM, FQKV], bf16, kind="ExternalInput")
    wproj_d = nc.dram_tensor("wproj", [DIM, DIM], bf16, kind="ExternalInput")
    bias_d = nc.dram_tensor("biasbf", [1, DIM], bf16, kind="ExternalInput")
    sin2_d = nc.dram_tensor("sin2", [P, 16 * HD], bf16, kind="ExternalInput")
    cos2_d = nc.dram_tensor("cos2", [P, 16 * HD], bf16, kind="ExternalInput")
    sneg2_d = nc.dram_tensor("sneg2", [P, 16 * HD], bf16, kind="ExternalInput")
    ident_d = nc.dram_tensor("ident", [P, P], bf16, kind="ExternalInput")
    out_d = nc.dram_tensor("out", [4 * P, DIM], f32, kind="ExternalOutput")
    a2a_in = {k: nc.dram_tensor(f"a2a_in{k[0]}_{k[1]}",
                                [NUM_CORES * P, tpc], bf16)
              for k, (_, _, tpc) in A2A_SPANS.items()}
    a2a_out = {k: nc.dram_tensor(f"a2a_out{k[0]}_{k[1]}",
                                 [NUM_CORES * P, tpc], bf16)
               for k, (_, _, tpc) in A2A_SPANS.items()}
    warm_in = nc.dram_tensor("warm_in", [P, 16], bf16)
    warm_out = nc.dram_tensor("warm_out", [P, 16], bf16)

    with tile.TileContext(nc) as tc:
        with (
            tc.tile_pool(name="persist", bufs=1) as pers,
            tc.tile_pool(name="work", bufs=3) as wp,
            tc.tile_pool(name="expp", bufs=4) as ep,
            tc.tile_pool(name="psA", bufs=2, space="PSUM") as psA,   # qkv/bc/proj/tp
            tc.tile_pool(name="psS", bufs=2, space="PSUM") as psS,   # scores
            tc.tile_pool(name="psV", bufs=1, space="PSUM") as psV,   # av accum
        ):
            # ---------------- persistent SBUF ----------------
            wqkvT = pers.tile([P, CT * FQKV], bf16)     # ct-block: [128c, 384f]
            wprojT = pers.tile([P, CT * DIM], bf16)     # dt-block: [128d', 1024f]
            xT = pers.tile([P, CT * T], bf16)           # ct-block: [128c, 4096t]
            qT = pers.tile([P, T], bf16)                # [d(2 heads), flat t]
            kT = pers.tile([P, T], bf16)
            v_sb = pers.tile([P, HPC * (T // P) * 65], bf16)
            aoT = pers.tile([P, T], bf16)               # [d', flat t]
            # A2A results: one tile per span so a later span's fetch never
            # creates a false write->read dependency for an earlier proj
            aoTr = {k: pers.tile([P, NUM_CORES * tpc], bf16,
                                 name=f"aoTr{k[0]}_{k[1]}",
                                 tag=f"aoTr{k[0]}_{k[1]}")
                    for k, (_, _, tpc) in A2A_SPANS.items()}
            sin2 = pers.tile([P, 16 * HD], bf16)
            cos2 = pers.tile([P, 16 * HD], bf16)
            sneg2 = pers.tile([P, 16 * HD], bf16)
            ones_col = pers.tile([1, P], bf16)
            bias_bf = pers.tile([1, DIM], bf16)
            ident = pers.tile([P, P], bf16)

            nc.vector.memset(ones_col, 1.0)
            nc.vector.memset(v_sb, 1.0)                 # ones cols survive

            # warm up the collectives path while DMAs stream in
            wtile = wp.tile([P, 16], bf16, tag="warm", bufs=1)
            nc.vector.memset(wtile, 0.0)
            nc.sync.dma_start(warm_in.ap(), wtile)
            nc.gpsimd.collective_compute(
                "AllToAll", mybir.AluOpType.bypass,
                replica_groups=[list(range(NUM_CORES))],
                ins=[warm_in.ap().opt()], outs=[warm_out.ap().opt()])

            # ---------------- prep loads (no xbar transposes) ----------------
            # scalar queue: wqkv, sincos, ident, bias (in need order);
            # sync queue: xT finest-grained first so tile-0's QKV matmuls
            # start as early as possible
            for ct in range(CT):
                nc.scalar.dma_start(wqkvT[:, FQKV * ct:FQKV * (ct + 1)],
                                    wqkv_d[P * ct:P * (ct + 1), :])
            nc.scalar.dma_start(sin2, sin2_d.ap())
            nc.scalar.dma_start(cos2, cos2_d.ap())
            nc.scalar.dma_start(sneg2, sneg2_d.ap())
            nc.scalar.dma_start(ident, ident_d.ap())
            nc.scalar.dma_start(bias_bf, bias_d.ap())
            for lo, hi in ((0, 128), (128, 512), (512, 1024),
                           (1024, 2048), (2048, 3072), (3072, 4096)):
                for ct in range(CT):
                    nc.sync.dma_start(
                        xT[:, T * ct + lo:T * ct + hi],
                        x_d[P * ct:P * (ct + 1), lo:hi])

            s1 = sin2.rearrange("p (pt c d) -> p pt c d", pt=16, c=1)
            c1 = cos2.rearrange("p (pt c d) -> p pt c d", pt=16, c=1)
            n1 = sneg2.rearrange("p (pt c d) -> p pt c d", pt=16, c=1)

            def bc4(src, like):
                """Broadcast a [p, 1, w] AP over the 4 q|k copies."""
                a, _ = broadcast_tensor_aps(src, like)
                return a

            def emit_qkv_tile(b, tt, act_copies=False):
                """QKV matmul + RoPE + PE transposes for one 128-token tile.

                act_copies: route PSUM->SBUF copies to ScalarE (only safe in
                windows where the exp stream has slack, i.e. b0's first
                q-block)."""
                cp = nc.scalar.copy if act_copies else nc.vector.tensor_copy
                ftt = TTH * b + tt
                qkvp = psA.tile([P, 512], f32, tag="mm", name="qkvp")
                for ct in range(CT):
                    base = T * ct + N * b
                    nc.tensor.matmul(
                        qkvp[:, 0:FQKV],
                        xT[:, base + P * tt:base + P * (tt + 1)],
                        wqkvT[:, FQKV * ct:FQKV * (ct + 1)],
                        start=(ct == 0), stop=(ct == CT - 1))
                qkc = wp.tile([P, 2 * QW], bf16, tag="qkc")
                cp(qkc, qkvp[:, 0:2 * QW])
                pt = tt % 16
                qk3 = qkc.rearrange("p (c d) -> p c d", c=4)
                t1 = wp.tile([P, 2 * QW], bf16, tag="t1")
                t13 = t1.rearrange("p (c d) -> p c d", c=4)
                nc.vector.tensor_tensor(
                    t13[:, :, 0:32], qk3[:, :, 32:64],
                    bc4(n1[:, pt, :, 0:32], qk3[:, :, 32:64]), mult)
                nc.vector.tensor_tensor(
                    t13[:, :, 32:64], qk3[:, :, 0:32],
                    bc4(n1[:, pt, :, 32:64], qk3[:, :, 0:32]), mult)
                qkcos = wp.tile([P, 2 * QW], bf16, tag="qkcos")
                qkcos3 = qkcos.rearrange("p (c d) -> p c d", c=4)
                nc.vector.tensor_tensor(
                    qkcos3, qk3, bc4(c1[:, pt, :, :], qk3), mult)
                qrope = wp.tile([P, QW], bf16, tag="qrope")
                krope = wp.tile([P, QW], bf16, tag="krope")
                nc.vector.tensor_tensor(qrope, qkcos[:, 0:QW], t1[:, 0:QW], add)
                nc.vector.tensor_tensor(krope, qkcos[:, QW:2 * QW],
                                        t1[:, QW:2 * QW], add)
                # in qq0 the scores slots are mostly idle; borrowing them
                # for the transpose tile lets qkvp double-buffer on psA
                tpool, ttag = (psS, "scores") if act_copies else (psA, "mm")
                tp = tpool.tile([P, 2 * P], bf16, tag=ttag, name="tp")
                nc.tensor.transpose(tp[:, 0:P], qrope, ident)
                nc.tensor.transpose(tp[:, P:2 * P], krope, ident)
                cp(qT[:, P * ftt:P * (ftt + 1)], tp[:, 0:P])
                cp(kT[:, P * ftt:P * (ftt + 1)], tp[:, P:2 * P])
                vv = v_sb.rearrange("p (h t e) -> p h t e", h=HPC, t=T // P)
                nc.vector.tensor_copy(
                    vv[:, :, ftt, 0:HD],
                    qkvp[:, 2 * QW:3 * QW].rearrange("p (h d) -> p h d", h=HPC))

            def emit_scores_exp(b, qq, jt):
                """Scores + exp for one (q-block, j-tile); returns the exp tile."""
                ftt = TTH * b + jt
                sp = psS.tile([P, HPC * QB], f32, tag="scores", name="sp")
                for h in range(HPC):
                    nc.tensor.matmul(
                        sp[:, QB * h:QB * (h + 1)],
                        kT[HD * h:HD * (h + 1), P * ftt:P * (ftt + 1)],
                        qT[HD * h:HD * (h + 1),
                           N * b + QB * qq:N * b + QB * (qq + 1)],
                        start=True, stop=True)
                et = ep.tile([P, HPC * QB], bf16, tag="expT", name="et")
                nc.scalar.activation(et, sp, Exp, scale=float(HD) ** -0.5)
                return et

            def emit_av(b, qq, jt, av, et):
                ftt = TTH * b + jt
                for h in range(HPC):
                    blk = (h * (T // P) + ftt) * 65
                    nc.tensor.matmul(av[h], v_sb[:, blk:blk + 65],
                                     et[:, QB * h:QB * (h + 1)],
                                     start=(jt == 0), stop=(jt == TTH - 1))

            def emit_avf(avp):
                avf = wp.tile([65, HPC * QB], f32, tag="avf", bufs=2, name="avf")
                nc.vector.tensor_copy(avf, avp)
                return avf

            def emit_norm_rest(b, qq, avf):
                """Denominator broadcast + reciprocal + normalize, plus the A2A
                staging of this q-block's chunks."""
                for h in range(HPC):
                    sums = wp.tile([1, QB], bf16, tag="sums", name="sums")
                    nc.vector.tensor_copy(sums, avf[64:65, QB * h:QB * (h + 1)])
                    bc = psA.tile([64, QB], f32, tag="mm", name="bc")
                    nc.tensor.matmul(bc, ones_col[:, 0:64], sums,
                                     start=True, stop=True)
                    rc = wp.tile([64, QB], f32, tag="recip", bufs=2, name="rc")
                    nc.vector.reciprocal_approx_fast(rc, bc)
                    nc.vector.tensor_tensor(
                        aoT[HD * h:HD * (h + 1),
                            N * b + QB * qq:N * b + QB * (qq + 1)],
                        avf[0:64, QB * h:QB * (h + 1)], rc, mult)
                # stage this q-block's chunks into its span's A2A input
                if b == 0 or qq < 2:
                    key, r0, c, t = (b, qq // 2), 4 * P * (qq % 2), 4, P
                else:
                    key, r0, c, t = (1, qq), 0, 8, 64
                a2i = a2a_in[key][r0:r0 + c * P].rearrange(
                    "(c p) t -> p c t", p=P)
                nc.sync.dma_start(
                    a2i, aoT[:, N * b + QB * qq:N * b + QB * (qq + 1)].rearrange(
                        "p (c t) -> p c t", c=c))

            def emit_a2a(key):
                nc.gpsimd.collective_compute(
                    "AllToAll", mybir.AluOpType.bypass,
                    replica_groups=[list(range(NUM_CORES))],
                    ins=[a2a_in[key].ap().opt()], outs=[a2a_out[key].ap().opt()])

            def emit_fetch(key):
                """Fetch A2A result into aoTr (GpSimd DMA queue: its wait on
                the collective must not block staging on the sync queue)."""
                a2o = a2a_out[key].ap().rearrange("(c p) t -> p c t", p=P)
                nc.gpsimd.dma_start(
                    aoTr[key].rearrange("p (c t) -> p c t", c=NUM_CORES), a2o)

            def make_proj_steps(key, act_ob=False):
                """Projection of one span, split into 8 small steps (2 matmuls
                each) so interleaving never starves the exp stream. act_ob:
                route the output copy to ScalarE (safe once exp is done)."""
                _, _, tpc = A2A_SPANS[key]
                rout = OUT_ROWS[key]
                cp = nc.scalar.copy if act_ob else nc.vector.tensor_copy
                st = {}
                steps = []

                def mk(fb, i):
                    def fn():
                        if i == 0:
                            st[fb] = psA.tile([tpc, 512], f32, tag="mm",
                                              name="proj")
                        for dt in (2 * i, 2 * i + 1):
                            lo = tpc * dt
                            nc.tensor.matmul(
                                st[fb], aoTr[key][:, lo:lo + tpc],
                                wprojT[:, DIM * dt + 512 * fb:
                                       DIM * dt + 512 * (fb + 1)],
                                start=(dt == 0), stop=False)
                        if i == 3:
                            nc.tensor.matmul(
                                st[fb], ones_col[:, 0:tpc],
                                bias_bf[:, 512 * fb:512 * (fb + 1)],
                                start=False, stop=True)
                            ob = wp.tile([tpc, 512], f32, tag="ob", bufs=2,
                                         name="ob")
                            cp(ob, st[fb])
                            nc.scalar.dma_start(
                                out_d[rout:rout + tpc,
                                      512 * fb:512 * (fb + 1)], ob)
                    return fn

                for fb in range(DIM // 512):
                    for i in range(4):
                        steps.append(mk(fb, i))
                return steps

            # ---------------- main schedule ----------------
            # b0: qq0 software-pipelines b0's QKV chain (ScalarE-assisted
            # copies — exp has slack there); qq1-3 interleave b1's QKV tiles
            # (DVE copies). AV matmuls trail scores/exp by one iteration and
            # the last AV + avf copy of each q-block is carried into the next
            # block's first iteration, so the boundary never stalls the exp
            # stream. Each span's A2A fires as soon as its q-blocks are
            # staged; its projection trickles into later q-blocks' PE slack.
            LAG = 4
            state = {"pend": None, "carry": None}
            proj_q = []

            def flush_carry():
                if state["carry"] is not None:
                    state["carry"]()
                    state["carry"] = None

            def set_carry(b, qq, avp, av, et):
                def fn():
                    emit_av(b, qq, TTH - 1, av, et)
                    state["pend"] = (b, qq, emit_avf(avp))
                state["carry"] = fn

            def flush_pend():
                if state["pend"] is not None:
                    emit_norm_rest(*state["pend"])
                    state["pend"] = None

            def proj_step():
                if proj_q:
                    proj_q.pop(0)()

            for qq in range(N // QB):
                avp = psV.tile([65, HPC * QB], f32, tag="av", name="avp")
                av = [avp[:, QB * h:QB * (h + 1)] for h in range(HPC)]
                prev_et = None
                if qq == 0:
                    for step in range(TTH + LAG):
                        if step < TTH:
                            emit_qkv_tile(0, step, act_copies=True)
                        if step >= LAG:
                            jt = step - LAG
                            et = emit_scores_exp(0, qq, jt)
                            if jt >= 1:
                                emit_av(0, qq, jt - 1, av, prev_et)
                            prev_et = et
                else:
                    for jt in range(TTH):
                        et = emit_scores_exp(0, qq, jt)
                        if jt == 0:
                            flush_carry()
                        else:
                            emit_av(0, qq, jt - 1, av, prev_et)
                        prev_et = et
                        if jt == 2:
                            flush_pend()
                            if qq == 2:
                                emit_a2a((0, 0))   # b0 half 0 staged by now
                            if qq == 3:
                                emit_fetch((0, 0))
                        if jt % 3 == 0:
                            nb1 = 6 * (qq - 1) + jt // 3
                            if nb1 < TTH:
                                emit_qkv_tile(1, nb1)
                set_carry(0, qq, avp, av, prev_et)
                if qq == 1:
                    for dt in range(CT):
                        nc.sync.dma_start(wprojT[:, DIM * dt:DIM * (dt + 1)],
                                          wproj_d[P * dt:P * (dt + 1), :])
            for qq in range(N // QB):
                avp = psV.tile([65, HPC * QB], f32, tag="av", name="avp")
                av = [avp[:, QB * h:QB * (h + 1)] for h in range(HPC)]
                prev_et = None
                for jt in range(TTH):
                    et = emit_scores_exp(1, qq, jt)
                    if jt == 0:
                        flush_carry()
                    else:
                        emit_av(1, qq, jt - 1, av, prev_et)
                    prev_et = et
                    if jt == 2:
                        flush_pend()
                        if qq == 0:
                            emit_a2a((0, 1))   # b0 half 1 staged by now
                            proj_q.extend(make_proj_steps((0, 0)))
                        if qq == 1:
                            proj_q.extend(make_proj_steps((0, 1)))
                        if qq == 2:
                            emit_a2a((1, 0))   # b1 half 0 staged by now
                        if qq == 3:
                            emit_a2a((1, 2))   # b1 quarter 2 staged by now
                            proj_q.extend(make_proj_steps((1, 0)))
                    if 4 <= jt <= 11:
                        proj_step()
                    if jt == 13:
                        if qq == 0:
                            emit_fetch((0, 1))
                        if qq == 2:
                            emit_fetch((1, 0))
                        if qq == 3:
                            emit_fetch((1, 2))
                set_carry(1, qq, avp, av, prev_et)
            flush_carry()
            flush_pend()
            emit_a2a((1, 3))
            for fn in proj_q:            # leftovers, if any
                fn()
            for fn in make_proj_steps((1, 2), act_ob=True):
                fn()
            emit_fetch((1, 3))
            for fn in make_proj_steps((1, 3), act_ob=True):
                fn()

    nc.compile()
    return nc


def _get_nc():
    if "nc" not in _CACHE:
        _CACHE["nc"] = _build_nc()
    return _CACHE["nc"]


def make_in_maps(x, Wqkv, Wproj, bproj, sin, cos):
    """Shard full (f32) inputs into per-core in_maps (pre-cast + pre-transposed)."""
    import ml_dtypes
    bf16 = ml_dtypes.bfloat16
    xT = np.ascontiguousarray(
        np.asarray(x, np.float32).reshape(T, DIM).astype(bf16).T)
    Wqkv = np.asarray(Wqkv, np.float32).astype(bf16)
    WprojT = np.ascontiguousarray(np.asarray(Wproj, np.float32).astype(bf16).T)
    biasbf = np.asarray(bproj, np.float32).reshape(1, DIM).astype(bf16)
    sin = np.asarray(sin, np.float32)
    cos = np.asarray(cos, np.float32)
    # sin/cos RoPE tiles, precomputed host-side:
    #   sin2[p, 64*pt + d] = sin[128*pt + p, d]
    #   sneg2 = -sin on d<32, +sin on d>=32 (rotate_half multiplier)
    s_t = sin.reshape(16, P, HD).transpose(1, 0, 2)          # [p, pt, d]
    c_t = cos.reshape(16, P, HD).transpose(1, 0, 2)
    n_t = np.concatenate([-s_t[..., 0:32], s_t[..., 32:64]], axis=-1)
    sin2 = np.ascontiguousarray(s_t.reshape(P, 16 * HD)).astype(bf16)
    cos2 = np.ascontiguousarray(c_t.reshape(P, 16 * HD)).astype(bf16)
    sneg2 = np.ascontiguousarray(n_t.reshape(P, 16 * HD)).astype(bf16)
    ident = np.eye(P, dtype=bf16)
    in_maps = []
    for c in range(NUM_CORES):
        r = P * c
        wq = Wqkv[r:r + P]
        wk = Wqkv[DIM + r:DIM + r + P]
        wv = Wqkv[2 * DIM + r:2 * DIM + r + P]
        in_maps.append({
            "x": xT,
            "wqkv": np.ascontiguousarray(np.concatenate([wq, wk, wv], 0).T),
            "wproj": WprojT,
            "biasbf": biasbf,
            "sin2": sin2,
            "cos2": cos2,
            "sneg2": sneg2,
            "ident": ident,
        })
    return in_maps


def assemble(outs):
    """Reassemble per-core 'out' tensors (row blocks per OUT_ROWS) into the
    full [B, N, DIM] output."""
    out = np.empty((B, N, DIM), np.float32)
    for c in range(NUM_CORES):
        o = outs[c]
        for key, (b, tok0, tpc) in A2A_SPANS.items():
            r = OUT_ROWS[key]
            t0 = tok0 + tpc * c
            out[b, t0:t0 + tpc] = o[r:r + tpc]
    return out


def kernel(x, Wqkv, Wproj, bproj, sin, cos):
    from concourse.bass_utils import run_bass_kernel_spmd

    nc = _get_nc()
    in_maps = make_in_maps(x, Wqkv, Wproj, bproj, sin, cos)
    trace = bool(int(os.environ.get("KERNEL_TRACE", "0")))
    res = run_bass_kernel_spmd(nc, in_maps, core_ids=list(range(NUM_CORES)),
                               trace=trace)
    _CACHE["last_result"] = res
    return assemble([res.results[c]["out"] for c in range(NUM_CORES)])
